# revision 1
# baseline (speedup 1.0000x reference)
"""Trainium2 Bass kernel for nn_MixtureOfExperts (B=524288, IN=59, E=4, H=64).

Strategy (pure data parallel over 8 cores, 65536 rows each):
 - Host folds BN into weights (scale into W, shift into per-feature bias),
   collapses expert head w3@wp -> wep (H->1), and pre-transposes x into a
   feature-on-partition layout so no on-chip transposes are needed.
 - On chip, everything is feature-major [feat, batch] with batch tiles of 512
   on the matmul moving dim.
 - Stage 1 + gating hidden run in float32r (full PE rate; ISA requires dst
   partition 0, so their outputs are full [128,*] tiles / zero-col-padded
   accumulations).  Stage 2 / preds / logits run in bf16 (dst partition can
   be 32-aligned, enabling strip packing + 4-way quadrant concurrency).
 - x is packed [128, S] with two independent 64-feature batch-halves on
   partition halves, so stage-1 matmuls for consecutive tiles land on
   disjoint PE row strips and overlap on the array.
 - Tiny outputs (gate hidden, logits, per-expert preds) are packed into full
   [128, 512] PSUM tiles via strips / zero-padded lhsT columns so the
   PSUM->SBUF hops always run with all 128 lanes busy.
 - softmax-weighted combine: pred = sum_e exp(l_e)*(p_e+b_e) / sum_e exp(l_e)
   (no max-subtraction needed; logits are O(1)).
"""

import numpy as np
import ml_dtypes

import concourse.bass as bass
import concourse.mybir as mybir
import concourse.tile as tile
from concourse import bacc
from concourse.bass_utils import run_bass_kernel_spmd

F32 = mybir.dt.float32
F32R = mybir.dt.float32r
BF16 = mybir.dt.bfloat16
AF = mybir.ActivationFunctionType
ALU = mybir.AluOpType

B, IN, E, H, EMB, GH = 524288, 59, 4, 64, 32, 32
EPS = 1e-5
NCORES = 8
BC = B // NCORES          # 65536 rows per core
S = 8192                  # rows per batch-half per superstep
SUP = BC // (2 * S)       # 4 supersteps
NT = (2 * S) // 512       # 32 tiles per superstep
BT = 512
W_F = 768 + 32 + 8        # f32r wts width: w1(256)+gate(512) | gsum | bias
W_B = 128 + 512 + 1024    # bf16 wts width: w2 | wep | gw2

_CACHE = {}


def _build():
    nc = bacc.Bacc(trn_type="TRN2")
    x_d = nc.dram_tensor("x", (SUP, 128, S), F32R, kind="ExternalInput")
    wts_d = nc.dram_tensor("wts", (128, W_F), F32R, kind="ExternalInput")
    wtsb_d = nc.dram_tensor("wtsb", (128, W_B), BF16, kind="ExternalInput")
    out_d = nc.dram_tensor("out", (SUP, NT, BT), F32, kind="ExternalOutput")

    with tile.TileContext(nc) as tc:
        with (
            tc.tile_pool(name="consts", bufs=1) as consts,
            tc.tile_pool(name="xp", bufs=2) as xp,
            tc.tile_pool(name="hs", bufs=2) as hs,
            tc.tile_pool(name="gts", bufs=2) as gts,
            tc.tile_pool(name="tails", bufs=2) as tails,
            tc.tile_pool(name="ph1a", bufs=1, space="PSUM") as ph1a,
            tc.tile_pool(name="ph1b", bufs=1, space="PSUM") as ph1b,
            tc.tile_pool(name="ph2a", bufs=1, space="PSUM") as ph2a,
            tc.tile_pool(name="ph2b", bufs=1, space="PSUM") as ph2b,
            tc.tile_pool(name="pga", bufs=1, space="PSUM") as pga,
            tc.tile_pool(name="pgb", bufs=1, space="PSUM") as pgb,
            tc.tile_pool(name="pl", bufs=1, space="PSUM") as pl,
            tc.tile_pool(name="pp", bufs=1, space="PSUM") as pp,
        ):
            wts_sb = consts.tile([128, W_F], F32R)
            nc.sync.dma_start(out=wts_sb, in_=wts_d[:, :])
            wtsb_sb = consts.tile([128, W_B], BF16)
            nc.sync.dma_start(out=wtsb_sb, in_=wtsb_d[:, :])
            w1_sb = wts_sb[:, 0:768]          # 0:256 experts, 256:768 gate(4x128)
            gs_sb = wts_sb[:, 768:800]
            bias_sb = wts_sb[:, 800:808].bitcast(F32)
            w2_sb = wtsb_sb[:, 0:128]
            wep_sb = wtsb_sb[:, 128:640].rearrange(
                "p (h j m) -> p h j m", h=2, j=8)
            gw2_sb = wtsb_sb[:, 640:1664].rearrange(
                "p (g j m) -> p g j m", g=4, j=8)
            c2a = bias_sb[:, 0:1]
            c2b = bias_sb[:, 1:2]
            gb2t = bias_sb[:, 2:3]
            bept = bias_sb[:, 3:4]

            for k in range(SUP):
                x_sb = xp.tile([128, S], F32R, tag="x")
                for ch in range(4):
                    cw = S // 4
                    nc.sync.dma_start(
                        out=x_sb[:, ch * cw : (ch + 1) * cw],
                        in_=x_d[k][:, ch * cw : (ch + 1) * cw])

                l_ps = pl.tile([128, BT], F32, tag="l")
                p_ps = pp.tile([128, BT], F32, tag="p")

                for q in range(4):            # group = pairs 4q..4q+3
                    # ---- gating: per-half accumulators (an f32r matmul
                    # group must keep one row base; mixing 0/64 into the
                    # same PSUM tile crashes the device).  4 zero-col-padded
                    # M=128 lhsT slots pack 4 tiles per [128,512] bank.
                    ga_ps = pga.tile([128, BT], F32, tag="ga")
                    gb_ps = pgb.tile([128, BT], F32, tag="gb")
                    for gi in range(4):
                        cg = (4 * q + gi) * BT
                        lt = w1_sb[:, 256 + 128 * gi : 384 + 128 * gi]
                        nc.tensor.matmul(
                            out=ga_ps,
                            lhsT=lt[0:64, :],
                            rhs=x_sb[0:64, cg : cg + BT],
                            start=(gi == 0), stop=(gi == 3),
                            skip_group_check=True,
                        )
                        nc.tensor.matmul(
                            out=gb_ps,
                            lhsT=lt[64:128, :],
                            rhs=x_sb[64:128, cg : cg + BT],
                            start=(gi == 0), stop=(gi == 3),
                            skip_group_check=True,
                        )
                    g1a_sb = gts.tile([128, BT], BF16, tag="g1a")
                    nc.scalar.activation(g1a_sb, ga_ps, AF.Relu)
                    g1b_sb = gts.tile([128, BT], BF16, tag="g1b")
                    nc.scalar.activation(g1b_sb, gb_ps, AF.Relu)

                    for pi in range(4):       # pair inside group
                        pr = 4 * q + pi
                        c0 = pr * BT
                        for half in (0, 1):
                            t = pr + 16 * half
                            base = 64 * half
                            strip = t // 8
                            j = t % 8
                            gslot = pi
                            g1_sb = g1a_sb if half == 0 else g1b_sb
                            xs = x_sb[base : base + 64, c0 : c0 + BT]

                            # ---- stage 1 (f32r, dst 0, M=128).  Biases are
                            # folded into the matmul via the ones-row of x
                            # (row 59 carries c1/gb1 in the weights).
                            h1a_ps = ph1a.tile([128, BT], F32, tag="h1a")
                            nc.tensor.matmul(
                                out=h1a_ps,
                                lhsT=w1_sb[base : base + 64, 0:128],
                                rhs=xs, start=True, stop=True,
                            )
                            h1b_ps = ph1b.tile([128, BT], F32, tag="h1b")
                            nc.tensor.matmul(
                                out=h1b_ps,
                                lhsT=w1_sb[base : base + 64, 128:256],
                                rhs=xs, start=True, stop=True,
                            )
                            h1a_sb = hs.tile([128, BT], BF16, tag="h1as")
                            nc.scalar.activation(h1a_sb, h1a_ps, AF.Relu)
                            h1b_sb = hs.tile([128, BT], BF16, tag="h1bs")
                            nc.vector.tensor_scalar(
                                h1b_sb, h1b_ps, 0.0, None, ALU.max)

                            # ---- stage 2 (bf16): 4 concurrent quadrants
                            h2a_ps = ph2a.tile([128, BT], F32, tag="h2a")
                            h2b_ps = ph2b.tile([128, BT], F32, tag="h2b")
                            nc.tensor.matmul(   # e0
                                out=h2a_ps[0:64, :], lhsT=w2_sb[0:64, 0:64],
                                rhs=h1a_sb[0:64, :], start=True, stop=True)
                            nc.tensor.matmul(   # e1
                                out=h2a_ps[64:128, :],
                                lhsT=w2_sb[64:128, 0:64],
                                rhs=h1a_sb[64:128, :], start=True, stop=True)
                            nc.tensor.matmul(   # e2 -> h2b[64:]
                                out=h2b_ps[64:128, :],
                                lhsT=w2_sb[0:64, 64:128],
                                rhs=h1b_sb[0:64, :], start=True, stop=True)
                            nc.tensor.matmul(   # e3 -> h2b[:64]
                                out=h2b_ps[0:64, :],
                                lhsT=w2_sb[64:128, 64:128],
                                rhs=h1b_sb[64:128, :], start=True, stop=True)
                            h2a_sb = hs.tile([128, BT], BF16, tag="h2as")
                            nc.scalar.activation(
                                h2a_sb, h2a_ps, AF.Relu, bias=c2a)
                            h2b_sb = hs.tile([128, BT], BF16, tag="h2bs")
                            nc.vector.tensor_scalar(
                                h2b_sb, h2b_ps, c2b, 0.0, ALU.add, ALU.max)

                            # ---- stage 3 (bf16): preds into p_ps strip
                            nc.tensor.matmul(
                                out=p_ps[32 * strip : 32 * strip + 32, :],
                                lhsT=wep_sb[:, 0, j, :],
                                rhs=h2a_sb,
                                start=(j == 0), stop=False,
                                skip_group_check=True,
                                tile_position=(0, 32 * strip),
                            )
                            nc.tensor.matmul(
                                out=p_ps[32 * strip : 32 * strip + 32, :],
                                lhsT=wep_sb[:, 1, j, :],
                                rhs=h2b_sb,
                                start=False, stop=(j == 7),
                                skip_group_check=True,
                                tile_position=(0, 32 * strip),
                            )

                            # ---- logits (bf16) into l_ps strip.  K=128
                            # with zero rows outside this gslot's strip so
                            # every mm in the accumulation group keeps row
                            # base 0 (mixed row bases crash the device).
                            nc.tensor.matmul(
                                out=l_ps[32 * strip : 32 * strip + 32, :],
                                lhsT=gw2_sb[:, gslot, j, :],
                                rhs=g1_sb,
                                start=(j == 0), stop=(j == 7),
                                skip_group_check=True,
                                tile_position=(0, 32 * strip),
                            )

                # ---- superstep tail (16384 rows), all full-width ops
                expl_sb = tails.tile([128, BT], F32R, tag="expl")
                nc.scalar.activation(expl_sb, l_ps, AF.Exp, bias=gb2t)
                pb_sb = tails.tile([128, BT], F32R, tag="pb")
                nc.vector.tensor_scalar(pb_sb, p_ps, bept, None, ALU.add)
                w_sb = tails.tile([128, BT], F32R, tag="wsb")
                nc.vector.tensor_mul(w_sb, pb_sb, expl_sb)

                num_ps = pl.tile([32, BT], F32, tag="l")
                nc.tensor.matmul(
                    out=num_ps, lhsT=gs_sb, rhs=w_sb, start=True, stop=True)
                den_ps = pp.tile([32, BT], F32, tag="p")
                nc.tensor.matmul(
                    out=den_ps, lhsT=gs_sb, rhs=expl_sb, start=True,
                    stop=True)
                denr_sb = tails.tile([32, BT], F32, tag="denr")
                out_sb = tails.tile([32, BT], F32, tag="outs")
                nc.vector.reciprocal(denr_sb, den_ps)
                nc.vector.tensor_mul(out_sb, num_ps, denr_sb)
                nc.sync.dma_start(out=out_d[k], in_=out_sb)

    if not nc.is_finalized():
        nc.finalize()
    return nc


def _pack_host(w1, b1, bn1_g, bn1_b, bn1_m, bn1_v, w2, b2, bn2_g, bn2_b,
               bn2_m, bn2_v, w3, b3, wp, bp, gw1, gb1, gw2, gb2):
    f = np.float32
    s1 = (bn1_g / np.sqrt(bn1_v + EPS)).astype(f)              # (E,H)
    w1e = (w1 * s1[:, None, :]).astype(f)                       # (E,IN,H)
    c1 = ((b1 - bn1_m) * s1 + bn1_b).astype(f)                  # (E,H)
    s2 = (bn2_g / np.sqrt(bn2_v + EPS)).astype(f)
    w2e = (w2 * s2[:, None, :]).astype(f)                       # (E,H,H)
    c2 = ((b2 - bn2_m) * s2 + bn2_b).astype(f)                  # (E,H)
    wep = np.einsum("ehm,em->eh", w3, wp).astype(f)             # (E,H)
    bep = (np.einsum("em,em->e", b3, wp) + bp).astype(f)        # (E,)

    # ---- f32r block: w1 experts (256) + gate 4 slots (4x128) + gsum + bias
    w1p = np.zeros((128, 768), f)
    half = np.zeros((64, 768), f)
    half[:IN, 0:64] = w1e[0]
    half[:IN, 64:128] = w1e[1]
    half[:IN, 128:192] = w1e[2]
    half[:IN, 192:256] = w1e[3]
    half[IN, 0:256] = np.concatenate([c1[0], c1[1], c1[2], c1[3]])
    for gi in range(4):
        lo = 256 + 128 * gi + 32 * gi
        half[:IN, lo : lo + 32] = gw1
        half[IN, lo : lo + 32] = gb1
    w1p[0:64] = half
    w1p[64:128] = half

    gsump = np.zeros((128, 32), f)
    for p in range(128):
        gsump[p, 8 * (p // 32) + (p % 32) // 4] = 1.0

    biasp = np.zeros((128, 8), f)
    biasp[:, 0] = np.concatenate([c2[0], c2[1]])
    biasp[:, 1] = np.concatenate([c2[3], c2[2]])   # h2b = [e3; e2]
    biasp[:, 2] = np.tile(gb2, 32)
    biasp[:, 3] = np.tile(bep, 32)

    wts = np.concatenate([w1p, gsump, biasp], axis=1)
    assert wts.shape == (128, W_F), wts.shape

    # ---- bf16 block: w2 quadrants + wep slots + gw2 slots
    w2p = np.zeros((128, 128), f)
    w2p[0:64, 0:64] = w2e[0]
    w2p[64:128, 0:64] = w2e[1]
    w2p[0:64, 64:128] = w2e[2]
    w2p[64:128, 64:128] = w2e[3]

    wepp = np.zeros((128, 2, 8, 32), f)
    for j in range(8):
        wepp[0:64, 0, j, 4 * j + 0] = wep[0]
        wepp[64:128, 0, j, 4 * j + 1] = wep[1]
        wepp[64:128, 1, j, 4 * j + 2] = wep[2]   # h2b = [e3; e2]
        wepp[0:64, 1, j, 4 * j + 3] = wep[3]

    gw2p = np.zeros((128, 4, 8, 32), f)
    for g in range(4):
        for j in range(8):
            gw2p[32 * g : 32 * g + 32, g, j, 4 * j : 4 * j + 4] = gw2

    wtsb = np.concatenate(
        [w2p, wepp.reshape(128, 512), gw2p.reshape(128, 1024)], axis=1)
    assert wtsb.shape == (128, W_B), wtsb.shape
    return dict(wts=np.ascontiguousarray(wts),
                wtsb=np.ascontiguousarray(wtsb.astype(ml_dtypes.bfloat16)))


def kernel(**inputs):
    x = np.asarray(inputs["x"], dtype=np.float32)
    wk = {k: np.asarray(v, dtype=np.float32) for k, v in inputs.items()
          if k != "x"}
    packed = _pack_host(**wk)

    if "nc" not in _CACHE:
        _CACHE["nc"] = _build()
    nc = _CACHE["nc"]

    in_maps = []
    for c in range(NCORES):
        xc = x[c * BC : (c + 1) * BC]                 # (BC, 59)
        xt = np.zeros((64, BC), np.float32)
        xt[:IN] = xc.T
        xt[IN] = 1.0
        xi = np.ascontiguousarray(
            xt.reshape(64, SUP, 2, S).transpose(1, 2, 0, 3).reshape(SUP, 128, S)
        )
        m = {"x": xi}
        m.update(packed)
        in_maps.append(m)

    res = run_bass_kernel_spmd(nc, in_maps, core_ids=list(range(NCORES)))
    _CACHE["last"] = res
    outs = [r["out"].reshape(BC) for r in res.results]
    return np.concatenate(outs).reshape(B, 1).astype(np.float32)



# revision 3
# speedup vs baseline: 1.9343x; 1.9343x over previous
"""Trainium2 Bass kernel for nn_MixtureOfExperts (B=524288, IN=59, E=4, H=64).

Data-parallel over 8 cores (65536 rows each). Per core, the batch is split
into two halves (A/B) carried on partition ranges 0:60 / 64:124 of a
feature-major x image, processed in 128 windows of 256 columns (512 rows).

Cost-model-driven design (CoreSim v1):
 - matmul cost = out-free-size only, so preds/logits/combine use "flipped"
   matmuls (data tile as stationary lhsT, +/-1 or gw2 patterns moving,
   N=2..4) which are nearly free.
 - stage-1 (5 mm, N=256, f32r) and stage-2 (4 mm bf16 block-diag, or 4
   DoubleRow fp8 mm at half cost) are the only bulk PE work.
 - every PSUM byte must be relu-evicted through Pool/Act/DVE (per-element
   engines); biases are folded into the matmuls (ones-row of x) or into
   the per-partition eviction scalar, and evictions are assigned to the
   three engines by a greedy load balancer at build time.
 - the per-row softmax-combine runs on batched [128,256] "smalls" PSUM
   tiles once per 8 windows: pattern-add (Pool), strided exp (Act),
   mult (Pool), segmented reduce + reciprocal + mul (DVE).
"""

import numpy as np
import ml_dtypes

import concourse.bass as bass
import concourse.mybir as mybir
import concourse.tile as tile
from concourse import bacc
from concourse.bass_utils import run_bass_kernel_spmd

F32 = mybir.dt.float32
F32R = mybir.dt.float32r
BF16 = mybir.dt.bfloat16
FP8 = mybir.dt.float8e4
AF = mybir.ActivationFunctionType
ALU = mybir.AluOpType
DRM = mybir.MatmulPerfMode.DoubleRow

B, IN, E, H, EMB, GH = 524288, 59, 4, 64, 32, 32
EPS = 1e-5
NCORES = 8
BC = B // NCORES            # 65536 rows per core
HB = BC // 2                # 32768 rows per half
WC = 256                    # x columns per window (= 256 A rows + 256 B rows)
NW = HB // WC               # 128 windows
CHW = 8                     # windows per x DMA chunk
NCH = NW // CHW             # 16 chunks
DRW = 8                     # windows per smalls drain
ND = NW // DRW              # 16 drains

VARIANT = "dr"              # "bf16" or "dr" (fp8 DoubleRow stage 2)

_CACHE = {}


def _build(variant):
    nc = bacc.Bacc(trn_type="TRN2")
    x_d = nc.dram_tensor("x", (128, HB), F32R, kind="ExternalInput")
    wts1_d = nc.dram_tensor("wts1", (128, 256), F32R, kind="ExternalInput")
    wg_d = nc.dram_tensor("wg", (128, 64), F32R, kind="ExternalInput")
    w2w = 256 if variant == "bf16" else 512
    w2dt = BF16 if variant == "bf16" else FP8
    h1dt = BF16 if variant == "bf16" else FP8
    w2b_d = nc.dram_tensor("w2b", (128, w2w), w2dt, kind="ExternalInput")
    c2s_d = nc.dram_tensor("c2s", (128, 2), F32, kind="ExternalInput")
    sp_d = nc.dram_tensor("sp", (128, 4), BF16, kind="ExternalInput")
    gw2_d = nc.dram_tensor("gw2t", (32, 8), BF16, kind="ExternalInput")
    pat_d = nc.dram_tensor("pat", (128, 256), F32, kind="ExternalInput")
    out_d = nc.dram_tensor("out", (ND, 128, 32), F32, kind="ExternalOutput")

    # greedy engine balancer for PSUM evictions (costs from the v1 model)
    load = {"pool": 0.0, "act": 0.0, "dve": 0.0}

    def evict(nc, out, in_, bias, cols):
        costs = {
            "pool": 0.833 * cols,
            "act": 0.833 * cols + 185.0,
            "dve": 1.0417 * cols + 125.0,
        }
        eng = min(costs, key=lambda e: load[e] + costs[e])
        load[eng] += costs[eng]
        if eng == "act":
            if bias is None:
                nc.scalar.activation(out, in_, AF.Relu)
            else:
                nc.scalar.activation(out, in_, AF.Relu, bias=bias)
        else:
            e = nc.gpsimd if eng == "pool" else nc.vector
            if bias is None:
                e.tensor_scalar(out, in_, 0.0, None, ALU.max)
            else:
                e.tensor_scalar(out, in_, bias, 0.0, ALU.add, ALU.max)

    with tile.TileContext(nc) as tc:
        with (
            tc.tile_pool(name="consts", bufs=1) as consts,
            tc.tile_pool(name="xs", bufs=2) as xs,
            tc.tile_pool(name="hs", bufs=2) as hs,
            tc.tile_pool(name="ds", bufs=2) as ds,
            tc.tile_pool(name="pha", bufs=2, space="PSUM") as pha,
            tc.tile_pool(name="phb", bufs=2, space="PSUM") as phb,
            tc.tile_pool(name="pg", bufs=2, space="PSUM") as pg,
            tc.tile_pool(name="psm", bufs=2, space="PSUM") as psm,
        ):
            wts1 = consts.tile([128, 256], F32R)
            nc.sync.dma_start(out=wts1, in_=wts1_d[:, :])
            wg = consts.tile([128, 64], F32R)
            nc.sync.dma_start(out=wg, in_=wg_d[:, :])
            w2b = consts.tile([128, w2w], w2dt)
            nc.sync.dma_start(out=w2b, in_=w2b_d[:, :])
            c2s = consts.tile([128, 2], F32)
            nc.sync.dma_start(out=c2s, in_=c2s_d[:, :])
            sp = consts.tile([128, 4], BF16)
            nc.sync.dma_start(out=sp, in_=sp_d[:, :])
            gw2 = consts.tile([32, 8], BF16)
            nc.sync.dma_start(out=gw2, in_=gw2_d[:, :])
            pat = consts.tile([128, 256], F32)
            nc.sync.dma_start(out=pat, in_=pat_d[:, :])

            state = {}
            xch = {}
            smt = {}
            for w in range(NW + 2):
                # ---- x chunk prefetch
                if w < NW and w % CHW == 0:
                    ci = w // CHW
                    xt = xs.tile([128, CHW * WC], F32R, tag="x")
                    nc.sync.dma_start(
                        out=xt, in_=x_d[:, ci * CHW * WC:(ci + 1) * CHW * WC])
                    xch[ci] = xt

                # ---- stage 1 + gating for window w
                if w < NW:
                    xt = xch[w // CHW]
                    lc = (w % CHW) * WC
                    xA = xt[0:60, lc:lc + WC]
                    xB = xt[64:124, lc:lc + WC]
                    pA = pha.tile([128, 512], F32, tag="hA")
                    nc.tensor.matmul(out=pA[:, 0:256], lhsT=wts1[0:60, 0:128],
                                     rhs=xA, start=True, stop=True,
                                     skip_group_check=True)
                    nc.tensor.matmul(out=pA[:, 256:512], lhsT=wts1[0:60, 128:256],
                                     rhs=xA, start=True, stop=True,
                                     skip_group_check=True)
                    pB = phb.tile([128, 512], F32, tag="hB")
                    nc.tensor.matmul(out=pB[:, 0:256], lhsT=wts1[64:124, 0:128],
                                     rhs=xB, start=True, stop=True,
                                     skip_group_check=True)
                    nc.tensor.matmul(out=pB[:, 256:512], lhsT=wts1[64:124, 128:256],
                                     rhs=xB, start=True, stop=True,
                                     skip_group_check=True)
                    pG = pg.tile([64, 256], F32, tag="g")
                    nc.tensor.matmul(out=pG, lhsT=wg[0:124, 0:64],
                                     rhs=xt[0:124, lc:lc + WC],
                                     start=True, stop=True,
                                     skip_group_check=True)
                    h1A = hs.tile([128, 512], h1dt, tag="h1A")
                    evict(nc, h1A, pA, None, 512)
                    h1B = hs.tile([128, 512], h1dt, tag="h1B")
                    evict(nc, h1B, pB, None, 512)
                    gsb = hs.tile([64, 256], BF16, tag="G")
                    evict(nc, gsb, pG, None, 256)
                    state[w] = (h1A, h1B, gsb, None, None)

                # ---- stage 2 for window w-1
                if 0 <= w - 1 < NW:
                    h1A, h1B, gsb, _, _ = state[w - 1]
                    p2a = pha.tile([128, 512], F32, tag="hA")
                    p2b = phb.tile([128, 512], F32, tag="hB")
                    if variant == "bf16":
                        for p2, wcol in ((p2a, slice(0, 128)), (p2b, slice(128, 256))):
                            hcol = slice(0, 256) if p2 is p2a else slice(256, 512)
                            nc.tensor.matmul(out=p2[:, 0:256], lhsT=w2b[:, wcol],
                                             rhs=h1A[:, hcol], start=True,
                                             stop=True, skip_group_check=True)
                            nc.tensor.matmul(out=p2[:, 256:512], lhsT=w2b[:, wcol],
                                             rhs=h1B[:, hcol], start=True,
                                             stop=True, skip_group_check=True)
                    else:
                        rA = h1A.rearrange("p (t n) -> p t n", t=2)
                        rB = h1B.rearrange("p (t n) -> p t n", t=2)
                        l02 = w2b[:, 0:256].rearrange("p (t m) -> p t m", t=2)
                        l13 = w2b[:, 256:512].rearrange("p (t m) -> p t m", t=2)
                        for p2, lw in ((p2a, l02), (p2b, l13)):
                            nc.tensor.matmul(out=p2[:, 0:256], lhsT=lw, rhs=rA,
                                             start=True, stop=True, perf_mode=DRM,
                                             skip_group_check=True)
                            nc.tensor.matmul(out=p2[:, 256:512], lhsT=lw, rhs=rB,
                                             start=True, stop=True, perf_mode=DRM,
                                             skip_group_check=True)
                    h2a = hs.tile([128, 512], BF16, tag="h2a")
                    evict(nc, h2a, p2a, c2s[:, 0:1], 512)
                    h2b = hs.tile([128, 512], BF16, tag="h2b")
                    evict(nc, h2b, p2b, c2s[:, 1:2], 512)
                    state[w - 1] = (h1A, h1B, gsb, h2a, h2b)

                # ---- preds/logits (flipped matmuls) for window w-2
                if 0 <= w - 2 < NW:
                    w2i = w - 2
                    _, _, gsb, h2a, h2b = state[w2i]
                    d = w2i // DRW
                    if w2i % DRW == 0:
                        sm_t = psm.tile([128, 256], F32, tag="sm")
                        smt[d] = sm_t
                    sm = smt[d]
                    for sl in range(4):
                        g0 = ((w2i % DRW) * 4 + sl) * 8
                        cl = 128 * sl
                        nc.tensor.matmul(out=sm[:, g0:g0 + 2],
                                         lhsT=h2a[:, cl:cl + 128],
                                         rhs=sp[:, 0:2], start=True, stop=True,
                                         skip_group_check=True,
                                         tile_position=(0, 0))
                        nc.tensor.matmul(out=sm[:, g0 + 2:g0 + 4],
                                         lhsT=h2b[:, cl:cl + 128],
                                         rhs=sp[:, 2:4], start=True, stop=True,
                                         skip_group_check=True,
                                         tile_position=(0, 0))
                        gp = 0 if sl < 2 else 32
                        gc = 128 * (sl % 2)
                        nc.tensor.matmul(out=sm[:, g0 + 4:g0 + 8],
                                         lhsT=gsb[gp:gp + 32, gc:gc + 128],
                                         rhs=gw2[0:32, 4 * (sl // 2):4 * (sl // 2) + 4],
                                         start=True, stop=True,
                                         skip_group_check=True,
                                         tile_position=(0, 0))
                    del state[w2i]

                    # ---- combine drain once per DRW windows
                    if w2i % DRW == DRW - 1:
                        S = ds.tile([128, 256], F32, tag="S")
                        nc.gpsimd.tensor_tensor(S, sm, pat, op=ALU.add)
                        S4 = S.rearrange("p (g two f) -> p g two f", two=2, f=4)
                        EX = ds.tile([128, 128], F32, tag="EX")
                        nc.scalar.activation(EX, S4[:, :, 1, :], AF.Exp)
                        PW = ds.tile([128, 128], F32, tag="PW")
                        nc.gpsimd.tensor_tensor(PW, S4[:, :, 0, :], EX, op=ALU.mult)
                        NUM = ds.tile([128, 32], F32, tag="NUM")
                        nc.vector.tensor_reduce(
                            NUM, PW.rearrange("p (g f) -> p g f", f=4),
                            mybir.AxisListType.X, ALU.add)
                        DEN = ds.tile([128, 32], F32, tag="DEN")
                        nc.vector.tensor_reduce(
                            DEN, EX.rearrange("p (g f) -> p g f", f=4),
                            mybir.AxisListType.X, ALU.add)
                        REC = ds.tile([128, 32], F32, tag="REC")
                        nc.vector.reciprocal(REC, DEN)
                        OUT = ds.tile([128, 32], F32, tag="OUT")
                        nc.vector.tensor_tensor(OUT, NUM, REC, op=ALU.mult)
                        nc.sync.dma_start(out=out_d[d], in_=OUT)
                        del smt[d]

    if not nc.is_finalized():
        nc.finalize()
    return nc


def _pack_host(variant, w1, b1, bn1_g, bn1_b, bn1_m, bn1_v, w2, b2, bn2_g,
               bn2_b, bn2_m, bn2_v, w3, b3, wp, bp, gw1, gb1, gw2, gb2):
    f = np.float32
    s1 = (bn1_g / np.sqrt(bn1_v + EPS)).astype(f)               # (E,H)
    W1e = (w1 * s1[:, None, :]).astype(f)                       # (E,IN,H)
    c1 = ((b1 - bn1_m) * s1 + bn1_b).astype(f)                  # (E,H)
    s2f = (bn2_g / np.sqrt(bn2_v + EPS)).astype(f)
    W2e = (w2 * s2f[:, None, :]).astype(f)                      # (E,H,H)
    c2 = ((b2 - bn2_m) * s2f + bn2_b).astype(f)                 # (E,H)
    wep = np.einsum("ehm,em->eh", w3, wp).astype(f)             # (E,H)
    bep = (np.einsum("em,em->e", b3, wp) + bp).astype(f)        # (E,)
    aw = np.abs(wep)
    sg = np.sign(wep).astype(f)
    W2p = (W2e * aw[:, None, :]).astype(f)                      # (E,H,H)
    c2p = (c2 * aw).astype(f)                                   # (E,H)

    wts1 = np.zeros((128, 256), f)
    for blk, (ea, eb) in enumerate(((0, 1), (2, 3))):
        c0 = 128 * blk
        for base in (0, 64):
            wts1[base:base + 59, c0:c0 + 64] = W1e[ea]
            wts1[base:base + 59, c0 + 64:c0 + 128] = W1e[eb]
            wts1[base + 59, c0:c0 + 64] = c1[ea]
            wts1[base + 59, c0 + 64:c0 + 128] = c1[eb]

    wgp = np.zeros((128, 64), f)
    wgp[0:59, 0:32] = gw1
    wgp[59, 0:32] = gb1
    wgp[64:123, 32:64] = gw1
    wgp[123, 32:64] = gb1

    if variant == "bf16":
        po = [0, 1, 2, 3]
        w2b = np.zeros((128, 256), f)
        w2b[0:64, 0:64] = W2p[0]
        w2b[64:128, 64:128] = W2p[1]
        w2b[0:64, 128:192] = W2p[2]
        w2b[64:128, 192:256] = W2p[3]
        c2sv = np.stack([np.concatenate([c2p[0], c2p[1]]),
                         np.concatenate([c2p[2], c2p[3]])], axis=1)
        spv = np.zeros((128, 4), f)
        spv[0:64, 0] = sg[0]
        spv[64:128, 1] = sg[1]
        spv[0:64, 2] = sg[2]
        spv[64:128, 3] = sg[3]
    else:
        po = [0, 2, 1, 3]
        t02 = np.zeros((128, 2, 128), f)
        t02[0:64, 0, 0:64] = W2p[0]
        t02[0:64, 1, 64:128] = W2p[2]
        t13 = np.zeros((128, 2, 128), f)
        t13[64:128, 0, 0:64] = W2p[1]
        t13[64:128, 1, 64:128] = W2p[3]
        w2b = np.concatenate([t02.reshape(128, 256), t13.reshape(128, 256)],
                             axis=1)
        c2sv = np.stack([np.concatenate([c2p[0], c2p[2]]),
                         np.concatenate([c2p[1], c2p[3]])], axis=1)
        spv = np.zeros((128, 4), f)
        spv[0:64, 0] = sg[0]
        spv[64:128, 1] = sg[2]
        spv[0:64, 2] = sg[1]
        spv[64:128, 3] = sg[3]

    gw2t = np.zeros((32, 8), f)
    gw2t[:, 0:4] = gw2[:, po]
    gw2t[:, 4:8] = gw2[:, po]

    grp = np.concatenate([bep[po], gb2[po]]).astype(f)          # (8,)
    patv = np.tile(grp, 32)[None, :].repeat(128, axis=0)

    w2dt = ml_dtypes.bfloat16 if variant == "bf16" else ml_dtypes.float8_e4m3fn
    return dict(
        wts1=np.ascontiguousarray(wts1),
        wg=np.ascontiguousarray(wgp),
        w2b=np.ascontiguousarray(w2b.astype(w2dt)),
        c2s=np.ascontiguousarray(c2sv.astype(f)),
        sp=np.ascontiguousarray(spv.astype(ml_dtypes.bfloat16)),
        gw2t=np.ascontiguousarray(gw2t.astype(ml_dtypes.bfloat16)),
        pat=np.ascontiguousarray(patv),
    )


def _pack_x_core(xc):
    # xc: (BC, 59) float32 -> feature-major image (128, HB)
    xi = np.zeros((128, HB), np.float32)
    xi[0:59] = xc[:HB].T
    xi[59] = 1.0
    xi[64:123] = xc[HB:].T
    xi[123] = 1.0
    return np.ascontiguousarray(xi)


def _unshard_core(o):
    # o: (ND, 128, 32) -> (BC,) predictions
    v = o.reshape(ND, 128, DRW, 4).transpose(0, 2, 3, 1)  # (d, wl, sl, p)
    a = v[:, :, 0:2, :].reshape(HB)
    b = v[:, :, 2:4, :].reshape(HB)
    return np.concatenate([a, b])


def kernel(**inputs):
    x = np.asarray(inputs["x"], dtype=np.float32)
    wk = {k: np.asarray(v, dtype=np.float32) for k, v in inputs.items()
          if k != "x"}
    packed = _pack_host(VARIANT, **wk)

    key = "nc_" + VARIANT
    if key not in _CACHE:
        _CACHE[key] = _build(VARIANT)
    nc = _CACHE[key]

    in_maps = []
    for c in range(NCORES):
        m = {"x": _pack_x_core(x[c * BC:(c + 1) * BC])}
        m.update(packed)
        in_maps.append(m)
    _CACHE["in_maps"] = in_maps

    res = run_bass_kernel_spmd(nc, in_maps, core_ids=list(range(NCORES)))
    _CACHE["last"] = res
    outs = [_unshard_core(r["out"]) for r in res.results]
    return np.concatenate(outs).reshape(B, 1).astype(np.float32)
